# revision 5
# baseline (speedup 1.0000x reference)
"""Trainium2 Bass kernel for the CNN-VAE loss:

    prob = einsum('klb,hwb->klhw', beta, A) * 5000
    mse  = mean(sum(|x - prob[:, :, None]|^2, axis=1))

Strategy
--------
K*L = 128 == SBUF partition count, so (k,l) lives on partitions and the
40000-pixel hw axis is sharded across the 8 cores (5000 pixels each);
every core sees all 128 (k,l) rows and all 3 channels of its hw slice.

Per core, per 500-pixel chunk:
  PE:   prob chunk = (5000*beta)^T .T @ A^T chunk   -> PSUM (128 x 500)
        (lhsT = scaled beta^T (3,128) stationary, rhs = A^T (3,500) moving)
  DVE:  d = x_c - prob          (tensor_tensor subtract, PSUM operand)
  ACT:  d2 = Square(d), accum_out -> per-partition partial sum column
Finally DVE reduces the 30 accum columns to (128,1), DMA'd out.

Host side: shard/transpose inputs, then sum the 8 per-core (128,)
partials and divide by 16*3*200*200 (the mean denominator; the sum over
L is folded into the partition-dim sum).
"""

import numpy as np

K, L, NB, H, W = 16, 8, 3, 200, 200
KL = K * L          # 128 partitions
C = 3               # broadcast channel dim of x
HW = H * W          # 40000
N_CORES = 8
HW_SHARD = HW // N_CORES   # 5000
CHUNK = 500
N_CHUNKS = HW_SHARD // CHUNK  # 10
SCALE = 5000.0
DENOM = float(K * C * H * W)  # mean over [K, C, H, W] after summing L

_NC = None


def _build():
    global _NC
    if _NC is not None:
        return _NC
    from contextlib import ExitStack

    import concourse.bacc as bacc
    import concourse.mybir as mybir
    import concourse.tile as tile

    f32 = mybir.dt.float32
    nc = bacc.Bacc("TRN2", target_bir_lowering=False, debug=False)

    xs = nc.dram_tensor("xs", [KL, C, HW_SHARD], f32, kind="ExternalInput").ap()
    at = nc.dram_tensor("at", [NB, HW_SHARD], f32, kind="ExternalInput").ap()
    bt = nc.dram_tensor("bt", [NB, KL], f32, kind="ExternalInput").ap()
    out = nc.dram_tensor("out", [KL, 1], f32, kind="ExternalOutput").ap()

    with tile.TileContext(nc) as tc, ExitStack() as ctx:
        const = ctx.enter_context(tc.tile_pool(name="const", bufs=1))
        xpool = ctx.enter_context(tc.tile_pool(name="x", bufs=4))
        dpool = ctx.enter_context(tc.tile_pool(name="d", bufs=4))
        d2pool = ctx.enter_context(tc.tile_pool(name="d2", bufs=2))
        ppool = ctx.enter_context(tc.tile_pool(name="prob", bufs=6, space="PSUM"))

        at_sb = const.tile([NB, HW_SHARD], f32)
        nc.sync.dma_start(at_sb[:], at[:])
        bt_sb = const.tile([NB, KL], f32)
        nc.sync.dma_start(bt_sb[:], bt[:])
        bts = const.tile([NB, KL], f32)
        nc.vector.tensor_scalar_mul(bts[:], bt_sb[:], SCALE)

        acc = const.tile([KL, C * N_CHUNKS], f32)

        for ci in range(N_CHUNKS):
            sl = slice(ci * CHUNK, (ci + 1) * CHUNK)
            pp = ppool.tile([KL, CHUNK], f32)
            nc.tensor.matmul(pp[:], bts[:], at_sb[:, sl], start=True, stop=True)
            xt = xpool.tile([KL, C, CHUNK], f32)
            nc.sync.dma_start(xt[:], xs[:, :, sl])
            for c in range(C):
                d = dpool.tile([KL, CHUNK], f32)
                nc.vector.tensor_sub(d[:], xt[:, c, :], pp[:])
                d2 = d2pool.tile([KL, CHUNK], f32)
                col = ci * C + c
                nc.scalar.activation(
                    d2[:],
                    d[:],
                    mybir.ActivationFunctionType.Square,
                    accum_out=acc[:, col : col + 1],
                )

        red = const.tile([KL, 1], f32)
        nc.vector.tensor_reduce(
            red[:], acc[:], axis=mybir.AxisListType.X, op=mybir.AluOpType.add
        )
        nc.sync.dma_start(out[:], red[:])

    nc.compile()
    _NC = nc
    return nc


def _make_in_maps(x, beta, A):
    x = np.ascontiguousarray(np.asarray(x, dtype=np.float32))
    beta = np.ascontiguousarray(np.asarray(beta, dtype=np.float32))
    A = np.ascontiguousarray(np.asarray(A, dtype=np.float32))

    xr = x.reshape(KL, C, HW)
    at_full = np.ascontiguousarray(A.reshape(HW, NB).T)  # (3, 40000)
    bt = np.ascontiguousarray(beta.reshape(KL, NB).T)    # (3, 128)

    in_maps = []
    for i in range(N_CORES):
        sl = slice(i * HW_SHARD, (i + 1) * HW_SHARD)
        in_maps.append(
            {
                "xs": np.ascontiguousarray(xr[:, :, sl]),
                "at": np.ascontiguousarray(at_full[:, sl]),
                "bt": bt,
            }
        )
    return in_maps


def _run(in_maps, trace=False, **kwargs):
    from concourse import bass_utils

    nc = _build()
    return bass_utils.run_bass_kernel_spmd(
        nc, in_maps, list(range(N_CORES)), trace=trace, **kwargs
    )


def _combine(results):
    total = 0.0
    for r in results:
        total += float(np.sum(np.asarray(r["out"], dtype=np.float64)))
    return np.float32(total / DENOM)


def kernel(x, beta, A):
    res = _run(_make_in_maps(x, beta, A))
    return _combine(res.results)


# revision 6
# speedup vs baseline: 1.0112x; 1.0112x over previous
"""Trainium2 Bass kernel for the CNN-VAE loss:

    prob = einsum('klb,hwb->klhw', beta, A) * 5000
    mse  = mean(sum(|x - prob[:, :, None]|^2, axis=1))

Strategy
--------
K*L = 128 == SBUF partition count, so (k,l) lives on partitions and the
40000-pixel hw axis is sharded across the 8 cores (5000 pixels each);
every core sees all 128 (k,l) rows and all 3 channels of its hw slice.

Per core:
  Phase 0 (overlapped with x DMA-in):
    PE:   prob = (5000*beta)^T .T @ A^T in 10 x 500-col fp32 matmuls
          (lhsT = scaled beta^T (3,128) stationary, rhs = A^T (3,500))
    DVE:  copy each PSUM bank into a persistent SBUF prob tile (128,5000)
  Steady state, 5 iterations of 1000 pixels x 3 channels:
    DVE:  d = x - prob  (one (128,3,1000) subtract; prob broadcast over
          the channel dim with a step-0 access pattern)
    ACT:  d2 = Square(d), accum_out -> per-partition partial sum column
  DVE reduces the 5 accum columns to (128,1), DMA'd out.

Host side: shard/transpose inputs, then sum the 8 per-core (128,)
partials and divide by 16*3*200*200 (the mean denominator; the sum over
L is folded into the partition-dim sum).
"""

import numpy as np

K, L, NB, H, W = 16, 8, 3, 200, 200
KL = K * L          # 128 partitions
C = 3               # broadcast channel dim of x
HW = H * W          # 40000
N_CORES = 8
HW_SHARD = HW // N_CORES   # 5000
MCHUNK = 500               # matmul chunk (one PSUM bank)
N_MCHUNKS = HW_SHARD // MCHUNK  # 10
XCHUNK = 1000              # steady-state pixels per iteration
N_X = HW_SHARD // XCHUNK   # 5
SCALE = 5000.0
DENOM = float(K * C * H * W)  # mean over [K, C, H, W] after summing L

_NC = None


def _build():
    global _NC
    if _NC is not None:
        return _NC
    from contextlib import ExitStack

    import concourse.bacc as bacc
    import concourse.mybir as mybir
    import concourse.tile as tile

    f32 = mybir.dt.float32
    nc = bacc.Bacc("TRN2", target_bir_lowering=False, debug=False)

    xs = nc.dram_tensor("xs", [KL, C, HW_SHARD], f32, kind="ExternalInput").ap()
    at = nc.dram_tensor("at", [NB, HW_SHARD], f32, kind="ExternalInput").ap()
    bt = nc.dram_tensor("bt", [NB, KL], f32, kind="ExternalInput").ap()
    out = nc.dram_tensor("out", [KL, 1], f32, kind="ExternalOutput").ap()

    with tile.TileContext(nc) as tc, ExitStack() as ctx:
        const = ctx.enter_context(tc.tile_pool(name="const", bufs=1))
        xpool = ctx.enter_context(tc.tile_pool(name="x", bufs=3))
        dpool = ctx.enter_context(tc.tile_pool(name="d", bufs=2))
        d2pool = ctx.enter_context(tc.tile_pool(name="d2", bufs=2))
        ppool = ctx.enter_context(tc.tile_pool(name="psum", bufs=8, space="PSUM"))

        at_sb = const.tile([NB, HW_SHARD], f32)
        nc.sync.dma_start(at_sb[:], at[:])
        bt_sb = const.tile([NB, KL], f32)
        nc.sync.dma_start(bt_sb[:], bt[:])
        bts = const.tile([NB, KL], f32)
        nc.vector.tensor_scalar_mul(bts[:], bt_sb[:], SCALE)

        # Phase 0: build prob in SBUF via back-to-back matmuls + DVE copies.
        prob_sb = const.tile([KL, HW_SHARD], f32)
        for ci in range(N_MCHUNKS):
            sl = slice(ci * MCHUNK, (ci + 1) * MCHUNK)
            pp = ppool.tile([KL, MCHUNK], f32)
            nc.tensor.matmul(pp[:], bts[:], at_sb[:, sl], start=True, stop=True)
            nc.vector.tensor_copy(prob_sb[:, sl], pp[:])

        # Steady state: subtract + square-accumulate in big fused tiles.
        acc = const.tile([KL, N_X], f32)
        for g in range(N_X):
            sl = slice(g * XCHUNK, (g + 1) * XCHUNK)
            xt = xpool.tile([KL, C, XCHUNK], f32)
            nc.sync.dma_start(xt[:], xs[:, :, sl])
            d = dpool.tile([KL, C, XCHUNK], f32)
            prob_b = prob_sb[:, sl].unsqueeze(1).broadcast_to([KL, C, XCHUNK])
            nc.vector.tensor_sub(d[:], xt[:], prob_b)
            d2 = d2pool.tile([KL, C, XCHUNK], f32)
            nc.scalar.activation(
                d2[:],
                d[:],
                mybir.ActivationFunctionType.Square,
                accum_out=acc[:, g : g + 1],
            )

        red = const.tile([KL, 1], f32)
        nc.vector.tensor_reduce(
            red[:], acc[:], axis=mybir.AxisListType.X, op=mybir.AluOpType.add
        )
        nc.sync.dma_start(out[:], red[:])

    nc.compile()
    _NC = nc
    return nc


def _make_in_maps(x, beta, A):
    x = np.ascontiguousarray(np.asarray(x, dtype=np.float32))
    beta = np.ascontiguousarray(np.asarray(beta, dtype=np.float32))
    A = np.ascontiguousarray(np.asarray(A, dtype=np.float32))

    xr = x.reshape(KL, C, HW)
    at_full = np.ascontiguousarray(A.reshape(HW, NB).T)  # (3, 40000)
    bt = np.ascontiguousarray(beta.reshape(KL, NB).T)    # (3, 128)

    in_maps = []
    for i in range(N_CORES):
        sl = slice(i * HW_SHARD, (i + 1) * HW_SHARD)
        in_maps.append(
            {
                "xs": np.ascontiguousarray(xr[:, :, sl]),
                "at": np.ascontiguousarray(at_full[:, sl]),
                "bt": bt,
            }
        )
    return in_maps


def _run(in_maps, trace=False, **kwargs):
    from concourse import bass_utils

    nc = _build()
    return bass_utils.run_bass_kernel_spmd(
        nc, in_maps, list(range(N_CORES)), trace=trace, **kwargs
    )


def _combine(results):
    total = 0.0
    for r in results:
        total += float(np.sum(np.asarray(r["out"], dtype=np.float64)))
    return np.float32(total / DENOM)


def kernel(x, beta, A):
    res = _run(_make_in_maps(x, beta, A))
    return _combine(res.results)
